# revision 37
# baseline (speedup 1.0000x reference)
"""ArcMargin head (ArcFace) distributed over 8 TRN2 NeuronCores.

Strategy (classification / tensor parallel), v8:
  - weight [C, D] sharded along C (12500 classes/core, padded to 12544);
    embeddings + labels replicated.  Weight is uploaded bf16 twice
    (transposed [D, CSP] for the matmul lhsT, natural [CSP, D] for the class
    norms); embeddings only once, transposed [D, B] (raw matmul rhs).
  - TRANSPOSED logits out[c, b] = 64 * (w_c . e_hat_b): classes sit on PSUM
    partitions, so both norms fold into PSUM evacuation: 1/||w_c|| is a
    per-partition scalar, 64/||e_b|| is the per-column tensor renb [128, B],
    built on device as ones[128,128] @ embt^2 (a K=128 matmul computes all
    column sums-of-squares broadcast to every partition) + sqrt + recip.
  - The first JD=6 chunks evacuate into SBUF staging with only the rn scale
    (renb is not ready yet) and are finalized a few chunks later - the
    TensorEngine never waits on the norm chain.
  - Output is bf16 (halves the dominant HBM write traffic; rel-err budget
    2e-2 >> bf16 noise).
  - ArcFace margin: the 64*cos values of the target classes already sit in
    the computed output, so the device gathers them back from out_d with 3
    indirect DMAs (the host sorts rows by target chunk so each gather only
    depends on chunks already written, with generous safety margins),
    applies the phi formula, and ships 64*phi out in a tiny tensor; the
    host places those values during unshard (indexing only).
"""

import math
import sys

import numpy as np
import ml_dtypes

for _p in ("/opt/trn_rl_repo",):
    if _p not in sys.path:
        sys.path.append(_p)

import concourse.bass as bass
import concourse.tile as tile
from concourse import bacc
from concourse import mybir
from concourse.bass_utils import run_bass_kernel_spmd

SCALE = 64.0
MARGIN = 0.5
COS_M = math.cos(MARGIN)
SIN_M = math.sin(MARGIN)
TH = math.cos(math.pi - MARGIN)
MM = math.sin(math.pi - MARGIN) * MARGIN

B, D, C = 2048, 512, 100000
N_CORES = 8
CS = C // N_CORES          # 12500 real classes per core
CSP = 12544                # padded classes per core (98 * 128)
NJ = CSP // 128            # 98 class chunks
CB = 1792                  # weight-block width (7 blocks x 14 chunks)
NBLK = CSP // CB           # 7
JPB = CB // 128            # 14 chunks per block
OOB = 1 << 30              # gather offset sentinel for "not my row"
JD = 8                     # chunks evacuated to SBUF staging (pre-renb)
NG = 3                     # phi gather columns
GJ0 = 70                   # emission chunk of gather 0 (bound: chunk < GJ0)

NPBF = ml_dtypes.bfloat16

F32 = mybir.dt.float32
BF16 = mybir.dt.bfloat16
I32 = mybir.dt.int32
AF = mybir.ActivationFunctionType
ALU = mybir.AluOpType


def build_program(b=B, d=D, csp=CSP):
    """Build the (SPMD-uniform) single-core Bass program."""
    kc = d // 128          # 4 contraction chunks
    nc = bacc.Bacc()

    embt_d = nc.declare_dram_parameter("embt", [d, b], BF16, isOutput=False)
    wt_d = nc.declare_dram_parameter("wt", [d, csp], BF16, isOutput=False)
    wn_d = nc.declare_dram_parameter("wn", [csp, d], BF16, isOutput=False)
    soff_d = nc.declare_dram_parameter("soff", [128, NG], I32, isOutput=False)
    # flat transposed output [c * B + b]
    out_d = nc.declare_dram_parameter("out", [csp * b, 1], BF16, isOutput=True)
    tv_d = nc.declare_dram_parameter("tv", [128, NG], F32, isOutput=True)

    with tile.TileContext(nc) as tc:
        with (
            tc.tile_pool(name="const", bufs=1) as constp,
            tc.tile_pool(name="persist", bufs=1) as persist,
            tc.tile_pool(name="wtp", bufs=3) as wtp,
            tc.tile_pool(name="wnp", bufs=3) as wnp,
            tc.tile_pool(name="scr", bufs=2) as scrp,
            tc.tile_pool(name="smp", bufs=4) as smp,
            tc.tile_pool(name="outp", bufs=4) as outp,
            tc.tile_pool(name="stg", bufs=1) as stgp,
            tc.tile_pool(name="cpsum", bufs=4, space="PSUM") as cpsum,
        ):
            zb = constp.tile([128, 1], F32, tag="zb")
            nc.vector.memset(zb[:], 0.0)
            epsb = constp.tile([128, 1], F32, tag="epsb")
            nc.vector.memset(epsb[:], 1e-24)
            s2b = constp.tile([128, 1], F32, tag="s2b")
            nc.vector.memset(s2b[:], SCALE * SCALE)
            onesb = constp.tile([128, 128], BF16, tag="onesb")
            nc.vector.memset(onesb[:], 1.0)

            embt = persist.tile([128, kc, b], BF16)     # e^T raw (matmul rhs)
            sqt = persist.tile([128, kc, b], BF16)      # embt^2
            renb = persist.tile([128, b], BF16)         # 64/||e_b|| bcast
            rsf = persist.tile([128, b], F32)           # 1/sum(e^2) scratch
            nsq = persist.tile([128, NJ], F32)          # per-class sum(w^2)
            nrm = persist.tile([128, NJ], F32)
            rn = persist.tile([128, NJ], F32)           # 1/||w_c||
            svec = persist.tile([128, NG], BF16)        # 64*cos(target), sorted
            tval = persist.tile([128, NG], F32)         # 64*phi, sorted
            sofft = persist.tile([128, NG], I32)
            stg = stgp.tile([128, JD, b], BF16)         # staged rn-scaled out

            outv = out_d[:].rearrange("(c b) o -> c (b o)", b=b)  # [csp, b]

            # ---------------- DMA helpers ----------------
            wt_tiles = {}

            def wt_blk(blk):
                t = wtp.tile([128, kc, CB], BF16, tag="wt", name=f"wt_{blk}")
                nc.sync.dma_start(
                    out=t[:],
                    in_=wt_d[:, blk * CB:(blk + 1) * CB].rearrange(
                        "(k p) c -> p k c", p=128
                    ),
                )
                wt_tiles[blk] = t

            wn_tiles = {}

            def wn_g(g):
                r0 = g * 512
                ng = min(4, NJ - g * 4)
                t = wnp.tile([128, 4, d], BF16, tag="wn", name=f"wn_{g}")
                nc.sync.dma_start(
                    out=t[:, :ng, :],
                    in_=wn_d[r0:r0 + ng * 128, :].rearrange(
                        "(g2 p) dd -> p g2 dd", p=128
                    ),
                )
                wn_tiles[g] = t

            # ---------------- compute helpers ----------------
            def wnorm_chunk(c):
                sq = scrp.tile([128, d], BF16, tag="sqw")
                nc.scalar.activation(
                    out=sq[:], in_=wn_tiles[c // 4][:, c % 4, :], func=AF.Square,
                    bias=zb[:], accum_out=nsq[:, c:c + 1],
                )

            def rn_fin(g):
                s0 = g * 4
                s1 = min(s0 + 4, NJ)
                nc.scalar.activation(
                    out=nrm[:, s0:s1], in_=nsq[:, s0:s1], func=AF.Sqrt, bias=epsb[:]
                )
                nc.vector.reciprocal(out=rn[:, s0:s1], in_=nrm[:, s0:s1])

            def phi_gather(q):
                nc.gpsimd.indirect_dma_start(
                    out=svec[:, q:q + 1],
                    out_offset=None,
                    in_=out_d[:],
                    in_offset=bass.IndirectOffsetOnAxis(
                        ap=sofft[:, q:q + 1], axis=0
                    ),
                    bounds_check=csp * b - 1,
                    oob_is_err=False,
                )

            def phi_block(q0, q1):
                nq = q1 - q0
                sv = svec[:, q0:q1]
                s2 = smp.tile([128, nq], F32, tag=f"s2{q0}")
                nc.vector.tensor_tensor(out=s2[:], in0=sv, in1=sv, op=ALU.mult)
                sn = smp.tile([128, nq], F32, tag=f"sn{q0}")
                # sin = sqrt(4096 - s^2); s^2 <= 4096 exactly (|cos| <= 1)
                nc.scalar.activation(
                    out=sn[:], in_=s2[:], func=AF.Sqrt, bias=s2b[:], scale=-1.0
                )
                pc = smp.tile([128, nq], F32, tag=f"pc{q0}")
                nc.vector.tensor_scalar_mul(out=pc[:], in0=sv, scalar1=COS_M)
                smt = smp.tile([128, nq], F32, tag=f"smt{q0}")
                nc.vector.tensor_scalar_mul(out=smt[:], in0=sn[:], scalar1=SIN_M)
                ph = smp.tile([128, nq], F32, tag=f"ph{q0}")
                nc.vector.tensor_tensor(
                    out=ph[:], in0=pc[:], in1=smt[:], op=ALU.subtract
                )
                eb = smp.tile([128, nq], F32, tag=f"eb{q0}")
                nc.vector.tensor_scalar_add(
                    out=eb[:], in0=sv, scalar1=-SCALE * MM
                )
                mk = smp.tile([128, nq], mybir.dt.uint8, tag=f"mk{q0}")
                nc.vector.tensor_scalar(
                    out=mk[:], in0=sv, scalar1=SCALE * TH, scalar2=None,
                    op0=ALU.is_gt,
                )
                nc.vector.select(
                    out=tval[:, q0:q1], mask=mk[:], on_true=ph[:], on_false=eb[:]
                )
                nc.sync.dma_start(out=tv_d[:, q0:q1], in_=tval[:, q0:q1])

            # ---------------- prologue (DMA order = ring order; order sets
            # the completion barrier each consumer waits on) ----------------
            nc.sync.dma_start(
                out=embt[:, :, 0:1024],
                in_=embt_d[:, 0:1024].rearrange("(k p) c -> p k c", p=128),
            )
            # first two lhsT chunks ride the empty SWDGE queue so the PE
            # start is gated by the embt half alone
            wt0a = wtp.tile([128, kc, 256], BF16, tag="wt0a")
            nc.gpsimd.dma_start(
                out=wt0a[:],
                in_=wt_d[:, 0:256].rearrange("(k p) c -> p k c", p=128),
            )
            wn_g(0)
            nc.sync.dma_start(
                out=embt[:, :, 1024:2048],
                in_=embt_d[:, 1024:2048].rearrange("(k p) c -> p k c", p=128),
            )
            wt0b = wtp.tile([128, kc, 768], BF16, tag="wt0b")
            nc.sync.dma_start(
                out=wt0b[:],
                in_=wt_d[:, 256:1024].rearrange("(k p) c -> p k c", p=128),
            )
            wn_g(1)
            wt0c = wtp.tile([128, kc, CB - 1024], BF16, tag="wt0c")
            nc.sync.dma_start(
                out=wt0c[:],
                in_=wt_d[:, 1024:CB].rearrange("(k p) c -> p k c", p=128),
            )
            wn_g(2)
            wt_blk(1)
            nc.sync.dma_start(out=sofft[:], in_=soff_d[:])

            wdone = 0
            while wdone < 8:
                wnorm_chunk(wdone)
                wdone += 1
                if wdone % 4 == 0:
                    rn_fin(wdone // 4 - 1)

            # ---------------- main loop over class chunks ----------------
            dsent = 0
            for j in range(NJ):
                blk, jj = divmod(j, JPB)
                # two-block weight prefetch (bufs=3: cur, +1, +2 in flight)
                if j == 0:
                    wt_blk(2)
                elif jj == 0 and 1 <= blk <= NBLK - 3:
                    wt_blk(blk + 2)
                if j % 4 == 0:
                    g = j // 4 + 3
                    if g * 4 < NJ:
                        wn_g(g)
                while wdone < min(NJ, j + 9):
                    wnorm_chunk(wdone)
                    wdone += 1
                    if wdone % 4 == 0 or wdone == NJ:
                        rn_fin((wdone - 1) // 4)

                if j == 2:
                    for k in range(kc):
                        nc.vector.tensor_tensor(
                            out=sqt[:, k, :], in0=embt[:, k, :],
                            in1=embt[:, k, :], op=ALU.mult,
                        )
                if j == 3:
                    # renb = 64/||e_b|| on every partition: ones-matmul
                    # computes column sums of embt^2 broadcast over partitions
                    for hh in range(2):
                        psr = cpsum.tile(
                            [128, 1024], F32, tag="mmps", name=f"ps_re{hh}"
                        )
                        for t2 in range(2):
                            for k in range(kc):
                                nc.tensor.matmul(
                                    out=psr[:, t2 * 512:(t2 + 1) * 512],
                                    lhsT=onesb[:],
                                    rhs=sqt[:, k, (2 * hh + t2) * 512:
                                            (2 * hh + t2 + 1) * 512],
                                    start=(k == 0),
                                    stop=(k == kc - 1),
                                )
                        # renb = 64/sqrt(ssq) = sqrt(4096 * (1/ssq)):
                        # fast approx reciprocal (18 bits >> bf16), then a
                        # scalar-engine sqrt with the scale folded in
                        nc.vector.reciprocal_approx_fast(
                            out=rsf[:, hh * 1024:(hh + 1) * 1024], in_=psr[:]
                        )
                        nc.scalar.activation(
                            out=renb[:, hh * 1024:(hh + 1) * 1024],
                            in_=rsf[:, hh * 1024:(hh + 1) * 1024],
                            func=AF.Sqrt, bias=epsb[:], scale=SCALE * SCALE,
                        )

                if blk == 0:
                    def lhs(k, jj=jj):
                        if jj < 2:
                            return wt0a[:, k, jj * 128:(jj + 1) * 128]
                        if jj < 8:
                            return wt0b[:, k, (jj - 2) * 128:(jj - 1) * 128]
                        return wt0c[:, k, (jj - 8) * 128:(jj - 7) * 128]
                else:
                    def lhs(k, jj=jj, cw=wt_tiles[blk]):
                        return cw[:, k, jj * 128:(jj + 1) * 128]

                if j == NJ - 1:
                    phi_gather(1)
                staged = j < JD
                ot = None if staged else outp.tile([128, b], BF16, tag="ot")
                for h in range(2):
                    ps = cpsum.tile([128, 1024], F32, tag="mmps")
                    for t in (2 * h, 2 * h + 1):
                        for k in range(kc):
                            nc.tensor.matmul(
                                out=ps[:, (t % 2) * 512:(t % 2) * 512 + 512],
                                lhsT=lhs(k),
                                rhs=embt[:, k, t * 512:(t + 1) * 512],
                                start=(k == 0),
                                stop=(k == kc - 1),
                            )
                    if staged:
                        # rn-only evacuation into SBUF staging (no renb dep)
                        if h == 0:
                            nc.vector.tensor_scalar_mul(
                                out=stg[:, j, 0:1024], in0=ps[:, :],
                                scalar1=rn[:, j:j + 1],
                            )
                        else:
                            nc.scalar.mul(
                                out=stg[:, j, 1024:2048], in_=ps[:, :],
                                mul=rn[:, j:j + 1],
                            )
                    elif h == 0:
                        # ot = (ps * rn[c]) * renb[b], fused on DVE
                        nc.vector.scalar_tensor_tensor(
                            out=ot[:, :1024], in0=ps[:, :], scalar=rn[:, j:j + 1],
                            in1=renb[:, 0:1024], op0=ALU.mult, op1=ALU.mult,
                        )
                    else:
                        ot1 = scrp.tile([128, 1024], BF16, tag="ot1")
                        nc.scalar.mul(
                            out=ot1[:], in_=ps[:, :], mul=rn[:, j:j + 1]
                        )
                        nc.vector.tensor_tensor(
                            out=ot[:, 1024:], in0=ot1[:], in1=renb[:, 1024:2048],
                            op=ALU.mult,
                        )
                if not staged:
                    nc.sync.dma_start(
                        out=outv[j * 128:(j + 1) * 128, :], in_=ot[:]
                    )

                # finalize one staged chunk every other j once renb exists
                if j >= 9 and j % 2 == 1 and dsent < JD:
                    jd = dsent
                    otd = outp.tile([128, b], BF16, tag="ot")
                    nc.vector.tensor_tensor(
                        out=otd[:, 0:1024], in0=stg[:, jd, 0:1024],
                        in1=renb[:, 0:1024], op=ALU.mult,
                    )
                    nc.vector.tensor_tensor(
                        out=otd[:, 1024:2048], in0=stg[:, jd, 1024:2048],
                        in1=renb[:, 1024:2048], op=ALU.mult,
                    )
                    nc.sync.dma_start(
                        out=outv[jd * 128:(jd + 1) * 128, :], in_=otd[:]
                    )
                    dsent += 1

                if j == GJ0:
                    phi_gather(0)

            # tail: gathers for the late chunks, phi, ship.  gather 1 was
            # emitted before chunk 97's DMA so it only waits on chunks <= 96
            phi_gather(2)
            phi_block(0, 2)
            phi_block(2, NG)

    nc.compile()
    return nc


_CACHE = {}


def _get_program():
    if "nc" not in _CACHE:
        _CACHE["nc"] = build_program()
    return _CACHE["nc"]


def make_in_maps(embeddings, labels, weight):
    emb = np.asarray(embeddings, dtype=np.float32)
    w = np.asarray(weight, dtype=np.float32)
    labels_np = np.asarray(labels).astype(np.int64)
    embt_bf = np.ascontiguousarray(emb.astype(NPBF).T)
    w_bf = w.astype(NPBF)
    in_maps = []
    perms = []
    for k in range(N_CORES):
        wn = np.zeros((CSP, D), NPBF)
        wn[:CS] = w_bf[k * CS:(k + 1) * CS]
        wT = np.ascontiguousarray(wn.T)
        own = (labels_np // CS) == k
        col = labels_np - k * CS
        chunk = np.where(own, col // 128, 1 << 20)
        order = np.argsort(chunk, kind="stable")
        # gather q=0 is emitted at chunk GJ0 (covers target chunks < GJ0,
        # incl. all staged chunks, finalized well before), q=1 before the
        # last chunk's write (covers chunks <= NJ-2), q=2 after everything.
        # Sorted owned rows fill columns while within bound; non-owned rows
        # (sorted last) pad the remainder with OOB offsets.
        bounds = [GJ0 - 1, NJ - 2, NJ - 1]
        cols = [[] for _ in range(NG)]
        spill = []
        q = 0
        for r in order:
            c = chunk[r]
            if c >= (1 << 20):
                spill.append(r)
                continue
            while q < NG - 1 and (c > bounds[q] or len(cols[q]) >= 128):
                q += 1
            cols[q].append(r)
        assert len(cols[NG - 1]) <= 128, "too many high-chunk targets"
        perm = np.full((128, NG), -1, np.int64)
        soff = np.full((128, NG), OOB, np.int64)
        si = 0
        for qq in range(NG):
            rows = list(cols[qq])
            while len(rows) < 128 and si < len(spill):
                rows.append(spill[si])
                si += 1
            for p, r in enumerate(rows):
                perm[p, qq] = r
                if own[r]:
                    soff[p, qq] = col[r] * B + r
        soff_arr = np.ascontiguousarray(soff.astype(np.int32))
        in_maps.append(
            {"embt": embt_bf, "wt": wT, "wn": wn, "soff": soff_arr}
        )
        perms.append(perm)
    return in_maps, perms


def _gather(results, labels, perms):
    labels_np = np.asarray(labels).astype(np.int64)
    fullT = np.empty((C, B), np.float32)
    for k in range(N_CORES):
        shard = np.asarray(results[k]["out"]).reshape(CSP, B)
        fullT[k * CS:(k + 1) * CS] = shard[:CS]
        # place the device-computed 64*phi values at the target positions
        tv = np.asarray(results[k]["tv"]).astype(np.float32)  # [128, NG]
        own = (labels_np // CS) == k
        perm = perms[k]
        pp, qq = np.nonzero(perm >= 0)
        rr = perm[pp, qq]
        sel = own[rr]
        fullT[labels_np[rr[sel]], rr[sel]] = tv[pp[sel], qq[sel]]
    return fullT.T


def kernel(embeddings, labels, weight):
    nc = _get_program()
    in_maps, perms = make_in_maps(embeddings, labels, weight)
    res = run_bass_kernel_spmd(nc, in_maps, core_ids=list(range(N_CORES)))
    return _gather(res.results, labels, perms)


def kernel_profiled(embeddings, labels, weight, **kw):
    """Like kernel() but also returns the BassKernelResults (exec_time_ns)."""
    nc = _get_program()
    in_maps, perms = make_in_maps(embeddings, labels, weight)
    res = run_bass_kernel_spmd(
        nc, in_maps, core_ids=list(range(N_CORES)), trace=True, **kw
    )
    return _gather(res.results, labels, perms), res


# revision 38
# speedup vs baseline: 1.1840x; 1.1840x over previous
"""ArcMargin head (ArcFace) distributed over 8 TRN2 NeuronCores.

Strategy (classification / tensor parallel), v8:
  - weight [C, D] sharded along C (12500 classes/core, padded to 12544);
    embeddings + labels replicated.  Weight is uploaded bf16 twice
    (transposed [D, CSP] for the matmul lhsT, natural [CSP, D] for the class
    norms); embeddings only once, transposed [D, B] (raw matmul rhs).
  - TRANSPOSED logits out[c, b] = 64 * (w_c . e_hat_b): classes sit on PSUM
    partitions, so both norms fold into PSUM evacuation: 1/||w_c|| is a
    per-partition scalar, 64/||e_b|| is the per-column tensor renb [128, B],
    built on device as ones[128,128] @ embt^2 (a K=128 matmul computes all
    column sums-of-squares broadcast to every partition) + sqrt + recip.
  - The first JD=6 chunks evacuate into SBUF staging with only the rn scale
    (renb is not ready yet) and are finalized a few chunks later - the
    TensorEngine never waits on the norm chain.
  - Output is bf16 (halves the dominant HBM write traffic; rel-err budget
    2e-2 >> bf16 noise).
  - ArcFace margin: the 64*cos values of the target classes already sit in
    the computed output, so the device gathers them back from out_d with 3
    indirect DMAs (the host sorts rows by target chunk so each gather only
    depends on chunks already written, with generous safety margins),
    applies the phi formula, and ships 64*phi out in a tiny tensor; the
    host places those values during unshard (indexing only).
"""

import math
import sys

import numpy as np
import ml_dtypes

for _p in ("/opt/trn_rl_repo",):
    if _p not in sys.path:
        sys.path.append(_p)

import concourse.bass as bass
import concourse.tile as tile
from concourse import bacc
from concourse import mybir
from concourse.bass_utils import run_bass_kernel_spmd

SCALE = 64.0
MARGIN = 0.5
COS_M = math.cos(MARGIN)
SIN_M = math.sin(MARGIN)
TH = math.cos(math.pi - MARGIN)
MM = math.sin(math.pi - MARGIN) * MARGIN

B, D, C = 2048, 512, 100000
N_CORES = 8
CS = C // N_CORES          # 12500 real classes per core
CSP = 12544                # padded classes per core (98 * 128)
NJ = CSP // 128            # 98 class chunks
CB = 1792                  # weight-block width (7 blocks x 14 chunks)
NBLK = CSP // CB           # 7
JPB = CB // 128            # 14 chunks per block
OOB = 1 << 30              # gather offset sentinel for "not my row"
JD = 8                     # chunks evacuated to SBUF staging (pre-renb)
NG = 3                     # phi gather columns
GJ0 = 70                   # emission chunk of gather 0 (bound: chunk < GJ0)

NPBF = ml_dtypes.bfloat16

F32 = mybir.dt.float32
BF16 = mybir.dt.bfloat16
I32 = mybir.dt.int32
AF = mybir.ActivationFunctionType
ALU = mybir.AluOpType


def build_program(b=B, d=D, csp=CSP):
    """Build the (SPMD-uniform) single-core Bass program."""
    kc = d // 128          # 4 contraction chunks
    nc = bacc.Bacc()

    embt_d = nc.declare_dram_parameter("embt", [d, b], BF16, isOutput=False)
    wt_d = nc.declare_dram_parameter("wt", [d, csp], BF16, isOutput=False)
    wn_d = nc.declare_dram_parameter("wn", [csp, d], BF16, isOutput=False)
    soff_d = nc.declare_dram_parameter("soff", [128, NG], I32, isOutput=False)
    # flat transposed output [c * B + b]
    out_d = nc.declare_dram_parameter("out", [csp * b, 1], BF16, isOutput=True)
    tv_d = nc.declare_dram_parameter("tv", [128, NG], F32, isOutput=True)

    with tile.TileContext(nc) as tc:
        with (
            tc.tile_pool(name="const", bufs=1) as constp,
            tc.tile_pool(name="persist", bufs=1) as persist,
            tc.tile_pool(name="wtp", bufs=3) as wtp,
            tc.tile_pool(name="wnp", bufs=3) as wnp,
            tc.tile_pool(name="scr", bufs=2) as scrp,
            tc.tile_pool(name="smp", bufs=4) as smp,
            tc.tile_pool(name="outp", bufs=4) as outp,
            tc.tile_pool(name="stg", bufs=1) as stgp,
            tc.tile_pool(name="cpsum", bufs=4, space="PSUM") as cpsum,
        ):
            zb = constp.tile([128, 1], F32, tag="zb")
            nc.vector.memset(zb[:], 0.0)
            epsb = constp.tile([128, 1], F32, tag="epsb")
            nc.vector.memset(epsb[:], 1e-24)
            s2b = constp.tile([128, 1], F32, tag="s2b")
            nc.vector.memset(s2b[:], SCALE * SCALE)
            onesb = constp.tile([128, 128], BF16, tag="onesb")
            nc.vector.memset(onesb[:], 1.0)

            embt = persist.tile([128, kc, b], BF16)     # e^T raw (matmul rhs)
            sqt = persist.tile([128, kc, b], BF16)      # embt^2
            renb = persist.tile([128, b], BF16)         # 64/||e_b|| bcast
            rsf = persist.tile([128, b], F32)           # 1/sum(e^2) scratch
            nsq = persist.tile([128, NJ], F32)          # per-class sum(w^2)
            nrm = persist.tile([128, NJ], F32)
            rn = persist.tile([128, NJ], F32)           # 1/||w_c||
            svec = persist.tile([128, NG], BF16)        # 64*cos(target), sorted
            tval = persist.tile([128, NG], F32)         # 64*phi, sorted
            sofft = persist.tile([128, NG], I32)
            stg = stgp.tile([128, JD, b], BF16)         # staged rn-scaled out

            outv = out_d[:].rearrange("(c b) o -> c (b o)", b=b)  # [csp, b]

            # ---------------- DMA helpers ----------------
            wt_tiles = {}

            def wt_blk(blk):
                t = wtp.tile([128, kc, CB], BF16, tag="wt", name=f"wt_{blk}")
                nc.sync.dma_start(
                    out=t[:],
                    in_=wt_d[:, blk * CB:(blk + 1) * CB].rearrange(
                        "(k p) c -> p k c", p=128
                    ),
                )
                wt_tiles[blk] = t

            wn_tiles = {}

            def wn_g(g):
                r0 = g * 512
                ng = min(4, NJ - g * 4)
                t = wnp.tile([128, 4, d], BF16, tag="wn", name=f"wn_{g}")
                nc.sync.dma_start(
                    out=t[:, :ng, :],
                    in_=wn_d[r0:r0 + ng * 128, :].rearrange(
                        "(g2 p) dd -> p g2 dd", p=128
                    ),
                )
                wn_tiles[g] = t

            # ---------------- compute helpers ----------------
            def wnorm_chunk(c):
                sq = scrp.tile([128, d], BF16, tag="sqw")
                nc.scalar.activation(
                    out=sq[:], in_=wn_tiles[c // 4][:, c % 4, :], func=AF.Square,
                    bias=zb[:], accum_out=nsq[:, c:c + 1],
                )

            def rn_fin(g):
                s0 = g * 4
                s1 = min(s0 + 4, NJ)
                nc.scalar.activation(
                    out=nrm[:, s0:s1], in_=nsq[:, s0:s1], func=AF.Sqrt, bias=epsb[:]
                )
                nc.vector.reciprocal(out=rn[:, s0:s1], in_=nrm[:, s0:s1])

            def phi_gather(q):
                nc.gpsimd.indirect_dma_start(
                    out=svec[:, q:q + 1],
                    out_offset=None,
                    in_=out_d[:],
                    in_offset=bass.IndirectOffsetOnAxis(
                        ap=sofft[:, q:q + 1], axis=0
                    ),
                    bounds_check=csp * b - 1,
                    oob_is_err=False,
                )

            def phi_block(q0, q1):
                nq = q1 - q0
                sv = svec[:, q0:q1]
                s2 = smp.tile([128, nq], F32, tag=f"s2{q0}")
                nc.vector.tensor_tensor(out=s2[:], in0=sv, in1=sv, op=ALU.mult)
                sn = smp.tile([128, nq], F32, tag=f"sn{q0}")
                # sin = sqrt(4096 - s^2); s^2 <= 4096 exactly (|cos| <= 1)
                nc.scalar.activation(
                    out=sn[:], in_=s2[:], func=AF.Sqrt, bias=s2b[:], scale=-1.0
                )
                pc = smp.tile([128, nq], F32, tag=f"pc{q0}")
                nc.vector.tensor_scalar_mul(out=pc[:], in0=sv, scalar1=COS_M)
                smt = smp.tile([128, nq], F32, tag=f"smt{q0}")
                nc.vector.tensor_scalar_mul(out=smt[:], in0=sn[:], scalar1=SIN_M)
                ph = smp.tile([128, nq], F32, tag=f"ph{q0}")
                nc.vector.tensor_tensor(
                    out=ph[:], in0=pc[:], in1=smt[:], op=ALU.subtract
                )
                eb = smp.tile([128, nq], F32, tag=f"eb{q0}")
                nc.vector.tensor_scalar_add(
                    out=eb[:], in0=sv, scalar1=-SCALE * MM
                )
                mk = smp.tile([128, nq], mybir.dt.uint8, tag=f"mk{q0}")
                nc.vector.tensor_scalar(
                    out=mk[:], in0=sv, scalar1=SCALE * TH, scalar2=None,
                    op0=ALU.is_gt,
                )
                nc.vector.select(
                    out=tval[:, q0:q1], mask=mk[:], on_true=ph[:], on_false=eb[:]
                )
                nc.sync.dma_start(out=tv_d[:, q0:q1], in_=tval[:, q0:q1])

            # ---------------- prologue (DMA order = ring order; order sets
            # the completion barrier each consumer waits on) ----------------
            nc.sync.dma_start(
                out=embt[:, :, 0:1024],
                in_=embt_d[:, 0:1024].rearrange("(k p) c -> p k c", p=128),
            )
            wt0a = wtp.tile([128, kc, 256], BF16, tag="wt0a")
            nc.sync.dma_start(
                out=wt0a[:],
                in_=wt_d[:, 0:256].rearrange("(k p) c -> p k c", p=128),
            )
            wn_g(0)
            nc.sync.dma_start(
                out=embt[:, :, 1024:2048],
                in_=embt_d[:, 1024:2048].rearrange("(k p) c -> p k c", p=128),
            )
            wt0b = wtp.tile([128, kc, 768], BF16, tag="wt0b")
            nc.sync.dma_start(
                out=wt0b[:],
                in_=wt_d[:, 256:1024].rearrange("(k p) c -> p k c", p=128),
            )
            wn_g(1)
            wt0c = wtp.tile([128, kc, CB - 1024], BF16, tag="wt0c")
            nc.sync.dma_start(
                out=wt0c[:],
                in_=wt_d[:, 1024:CB].rearrange("(k p) c -> p k c", p=128),
            )
            wn_g(2)
            wt_blk(1)
            nc.sync.dma_start(out=sofft[:], in_=soff_d[:])

            wdone = 0
            while wdone < 8:
                wnorm_chunk(wdone)
                wdone += 1
                if wdone % 4 == 0:
                    rn_fin(wdone // 4 - 1)

            # ---------------- main loop over class chunks ----------------
            dsent = 0
            for j in range(NJ):
                blk, jj = divmod(j, JPB)
                # two-block weight prefetch (bufs=3: cur, +1, +2 in flight)
                if j == 0:
                    wt_blk(2)
                elif jj == 0 and 1 <= blk <= NBLK - 3:
                    wt_blk(blk + 2)
                if j % 4 == 0:
                    g = j // 4 + 3
                    if g * 4 < NJ:
                        wn_g(g)
                while wdone < min(NJ, j + 9):
                    wnorm_chunk(wdone)
                    wdone += 1
                    if wdone % 4 == 0 or wdone == NJ:
                        rn_fin((wdone - 1) // 4)

                if j == 2:
                    for k in range(kc):
                        nc.vector.tensor_tensor(
                            out=sqt[:, k, :], in0=embt[:, k, :],
                            in1=embt[:, k, :], op=ALU.mult,
                        )
                if j == 3:
                    # renb = 64/||e_b|| on every partition: ones-matmul
                    # computes column sums of embt^2 broadcast over partitions
                    for hh in range(2):
                        psr = cpsum.tile(
                            [128, 1024], F32, tag="mmps", name=f"ps_re{hh}"
                        )
                        for t2 in range(2):
                            for k in range(kc):
                                nc.tensor.matmul(
                                    out=psr[:, t2 * 512:(t2 + 1) * 512],
                                    lhsT=onesb[:],
                                    rhs=sqt[:, k, (2 * hh + t2) * 512:
                                            (2 * hh + t2 + 1) * 512],
                                    start=(k == 0),
                                    stop=(k == kc - 1),
                                )
                        # renb = 64/sqrt(ssq) = sqrt(4096 * (1/ssq)):
                        # fast approx reciprocal (18 bits >> bf16), then a
                        # scalar-engine sqrt with the scale folded in
                        nc.vector.reciprocal_approx_fast(
                            out=rsf[:, hh * 1024:(hh + 1) * 1024], in_=psr[:]
                        )
                        nc.scalar.activation(
                            out=renb[:, hh * 1024:(hh + 1) * 1024],
                            in_=rsf[:, hh * 1024:(hh + 1) * 1024],
                            func=AF.Sqrt, bias=epsb[:], scale=SCALE * SCALE,
                        )

                if blk == 0:
                    def lhs(k, jj=jj):
                        if jj < 2:
                            return wt0a[:, k, jj * 128:(jj + 1) * 128]
                        if jj < 8:
                            return wt0b[:, k, (jj - 2) * 128:(jj - 1) * 128]
                        return wt0c[:, k, (jj - 8) * 128:(jj - 7) * 128]
                else:
                    def lhs(k, jj=jj, cw=wt_tiles[blk]):
                        return cw[:, k, jj * 128:(jj + 1) * 128]

                if j == NJ - 1:
                    phi_gather(1)
                staged = j < JD
                ot = None if staged else outp.tile([128, b], BF16, tag="ot")
                for h in range(2):
                    ps = cpsum.tile([128, 1024], F32, tag="mmps")
                    for t in (2 * h, 2 * h + 1):
                        for k in range(kc):
                            nc.tensor.matmul(
                                out=ps[:, (t % 2) * 512:(t % 2) * 512 + 512],
                                lhsT=lhs(k),
                                rhs=embt[:, k, t * 512:(t + 1) * 512],
                                start=(k == 0),
                                stop=(k == kc - 1),
                            )
                    if staged:
                        # rn-only evacuation into SBUF staging (no renb dep)
                        if h == 0:
                            nc.vector.tensor_scalar_mul(
                                out=stg[:, j, 0:1024], in0=ps[:, :],
                                scalar1=rn[:, j:j + 1],
                            )
                        else:
                            nc.scalar.mul(
                                out=stg[:, j, 1024:2048], in_=ps[:, :],
                                mul=rn[:, j:j + 1],
                            )
                    elif h == 0:
                        # ot = (ps * rn[c]) * renb[b], fused on DVE
                        nc.vector.scalar_tensor_tensor(
                            out=ot[:, :1024], in0=ps[:, :], scalar=rn[:, j:j + 1],
                            in1=renb[:, 0:1024], op0=ALU.mult, op1=ALU.mult,
                        )
                    else:
                        ot1 = scrp.tile([128, 1024], BF16, tag="ot1")
                        nc.scalar.mul(
                            out=ot1[:], in_=ps[:, :], mul=rn[:, j:j + 1]
                        )
                        nc.vector.tensor_tensor(
                            out=ot[:, 1024:], in0=ot1[:], in1=renb[:, 1024:2048],
                            op=ALU.mult,
                        )
                if not staged:
                    nc.sync.dma_start(
                        out=outv[j * 128:(j + 1) * 128, :], in_=ot[:]
                    )

                # finalize one staged chunk every other j once renb exists
                if j >= 9 and j % 2 == 1 and dsent < JD:
                    jd = dsent
                    otd = outp.tile([128, b], BF16, tag="ot")
                    nc.vector.tensor_tensor(
                        out=otd[:, 0:1024], in0=stg[:, jd, 0:1024],
                        in1=renb[:, 0:1024], op=ALU.mult,
                    )
                    nc.vector.tensor_tensor(
                        out=otd[:, 1024:2048], in0=stg[:, jd, 1024:2048],
                        in1=renb[:, 1024:2048], op=ALU.mult,
                    )
                    nc.sync.dma_start(
                        out=outv[jd * 128:(jd + 1) * 128, :], in_=otd[:]
                    )
                    dsent += 1

                if j == GJ0:
                    phi_gather(0)

            # tail: gathers for the late chunks, phi, ship.  gather 1 was
            # emitted before chunk 97's DMA so it only waits on chunks <= 96
            phi_gather(2)
            phi_block(0, 2)
            phi_block(2, NG)

    nc.compile()
    return nc


_CACHE = {}


def _get_program():
    if "nc" not in _CACHE:
        _CACHE["nc"] = build_program()
    return _CACHE["nc"]


def make_in_maps(embeddings, labels, weight):
    emb = np.asarray(embeddings, dtype=np.float32)
    w = np.asarray(weight, dtype=np.float32)
    labels_np = np.asarray(labels).astype(np.int64)
    embt_bf = np.ascontiguousarray(emb.astype(NPBF).T)
    w_bf = w.astype(NPBF)
    in_maps = []
    perms = []
    for k in range(N_CORES):
        wn = np.zeros((CSP, D), NPBF)
        wn[:CS] = w_bf[k * CS:(k + 1) * CS]
        wT = np.ascontiguousarray(wn.T)
        own = (labels_np // CS) == k
        col = labels_np - k * CS
        chunk = np.where(own, col // 128, 1 << 20)
        order = np.argsort(chunk, kind="stable")
        # gather q=0 is emitted at chunk GJ0 (covers target chunks < GJ0,
        # incl. all staged chunks, finalized well before), q=1 before the
        # last chunk's write (covers chunks <= NJ-2), q=2 after everything.
        # Sorted owned rows fill columns while within bound; non-owned rows
        # (sorted last) pad the remainder with OOB offsets.
        bounds = [GJ0 - 1, NJ - 2, NJ - 1]
        cols = [[] for _ in range(NG)]
        spill = []
        q = 0
        for r in order:
            c = chunk[r]
            if c >= (1 << 20):
                spill.append(r)
                continue
            while q < NG - 1 and (c > bounds[q] or len(cols[q]) >= 128):
                q += 1
            cols[q].append(r)
        assert len(cols[NG - 1]) <= 128, "too many high-chunk targets"
        perm = np.full((128, NG), -1, np.int64)
        soff = np.full((128, NG), OOB, np.int64)
        si = 0
        for qq in range(NG):
            rows = list(cols[qq])
            while len(rows) < 128 and si < len(spill):
                rows.append(spill[si])
                si += 1
            for p, r in enumerate(rows):
                perm[p, qq] = r
                if own[r]:
                    soff[p, qq] = col[r] * B + r
        soff_arr = np.ascontiguousarray(soff.astype(np.int32))
        in_maps.append(
            {"embt": embt_bf, "wt": wT, "wn": wn, "soff": soff_arr}
        )
        perms.append(perm)
    return in_maps, perms


def _gather(results, labels, perms):
    labels_np = np.asarray(labels).astype(np.int64)
    fullT = np.empty((C, B), np.float32)
    for k in range(N_CORES):
        shard = np.asarray(results[k]["out"]).reshape(CSP, B)
        fullT[k * CS:(k + 1) * CS] = shard[:CS]
        # place the device-computed 64*phi values at the target positions
        tv = np.asarray(results[k]["tv"]).astype(np.float32)  # [128, NG]
        own = (labels_np // CS) == k
        perm = perms[k]
        pp, qq = np.nonzero(perm >= 0)
        rr = perm[pp, qq]
        sel = own[rr]
        fullT[labels_np[rr[sel]], rr[sel]] = tv[pp[sel], qq[sel]]
    return fullT.T


def kernel(embeddings, labels, weight):
    nc = _get_program()
    in_maps, perms = make_in_maps(embeddings, labels, weight)
    res = run_bass_kernel_spmd(nc, in_maps, core_ids=list(range(N_CORES)))
    return _gather(res.results, labels, perms)


def kernel_profiled(embeddings, labels, weight, **kw):
    """Like kernel() but also returns the BassKernelResults (exec_time_ns)."""
    nc = _get_program()
    in_maps, perms = make_in_maps(embeddings, labels, weight)
    res = run_bass_kernel_spmd(
        nc, in_maps, core_ids=list(range(N_CORES)), trace=True, **kw
    )
    return _gather(res.results, labels, perms), res
